# revision 9
# baseline (speedup 1.0000x reference)
"""CNF (continuous normalizing flow) RK4 kernel for 8 Trainium2 NeuronCores.

Computes z, log_det = RK4-integrate of
    dx/dt = f(x,t) = MLP(x,t),   d(log_det)/dt = -trace(df/dx)
over t in [0,1] with 8 fixed steps, matching reference.py.

Key algebra: the exact Jacobian trace of the MLP collapses to
    tr = g1^T (W2 * C^T) g2,   C = W3 @ W1[:D],  g_i = 1 - h_i^2
so no [B,D,D] Jacobian is ever materialized -- one extra HxH matmul per
RK stage ("trace matmul") replaces the jacfwd.

Sharding: pure data parallel, batch 512 -> 8 cores x 64 rows.

Per-core dataflow (per RK stage), engine-balanced:
  MM1 (weights stationary) -> a1T feature-major [H,B] chunks in PSUM
  tanh (+b1 as per-partition ACT bias) -> h1T (fp32r)
  s1T=h1T^2 ; g1wT = w*(1-s1T)  (fp32r, trace stationary)
  MM2: h1T stationary, W2 moving (fp32r, N=512) -> a2 [B,H]
  tanh -> h2 (bf16); PE-transpose -> h2T (L3 stationary-side input)
  trace: g1wT stationary, M moving -> y' = w*g1^T M  [B,H] PSUM
  s2 = h2^2 ; fused scalar_tensor_tensor: sum((s2-1)*y') -> -w*tr
  per-stage -w*tr lands in a [B,32] buffer; one reduce at the end -> ld
  L3: RK4-prescaled W3 variants stationary, h2T moving -> dxT increments
      accumulated in PSUM (S per-stage / F per-step), state += in fp32
"""

import numpy as np

_B, _D, _H = 512, 64, 512
_NSTEPS = 8
_NCORES = 8
_BL = _B // _NCORES  # 64 batch rows per core
_DT = 1.0 / _NSTEPS

_compiled = {}

# scheduling knobs
_TUNE = {"pa": 2, "S": 1, "F": 1, "wp": 2, "tp": 2, "sp": 2}


def _build_nc(use_bf16, with_b1, with_b2, with_b3, reps, tune=None):
    T = dict(_TUNE)
    if tune:
        T.update(tune)
    import concourse.bacc as bacc
    import concourse.mybir as mybir
    import concourse.tile as tile

    dt = mybir.dt
    f32, f32r, bf16 = dt.float32, dt.float32r, dt.bfloat16
    tdt = bf16 if use_bf16 else f32r  # h2 / L3 dtype
    AF = mybir.ActivationFunctionType
    OP = mybir.AluOpType

    D, H, BL = _D, _H, _BL
    KU = D + 1  # u = [x, t]
    NCH = H // 128  # 4 k-chunks

    # per-stage RK coefficients
    S_VAR = [0, 0, 1]            # W3 scale variant for state increments
    F_VAR = [2, 3, 3, 2]         # W3 scale variant for the final sum
    LD_W = [_DT / 6.0, _DT / 3.0, _DT / 3.0, _DT / 6.0]

    nc = bacc.Bacc("TRN2", target_bir_lowering=False, debug=False,
                   num_devices=_NCORES)

    u0_d = nc.dram_tensor("u0", [KU, BL], f32r, kind="ExternalInput")
    W1e_d = nc.dram_tensor("W1e", [KU, H], f32r, kind="ExternalInput")
    if with_b1:
        b1c_d = nc.dram_tensor("b1c", [128, NCH], f32, kind="ExternalInput")
    W2s_d = nc.dram_tensor("W2s", [128, NCH, H], f32r, kind="ExternalInput")
    Ms_d = nc.dram_tensor("Ms", [128, NCH, H], f32r, kind="ExternalInput")
    W3p_d = nc.dram_tensor("W3p", [128, NCH, 4, D], tdt, kind="ExternalInput")
    idb_d = nc.dram_tensor("idb", [BL, BL], tdt, kind="ExternalInput")
    trows_d = nc.dram_tensor("trows", [1, 17 * BL], f32r, kind="ExternalInput")
    if with_b2:
        b2r_d = nc.dram_tensor("b2r", [1, H], f32r, kind="ExternalInput")
        onesr_d = nc.dram_tensor("onesr", [1, BL], f32r, kind="ExternalInput")
    if with_b3:
        b3p_d = nc.dram_tensor("b3p", [1, 4, D], tdt, kind="ExternalInput")
        onesb_d = nc.dram_tensor("onesb", [1, BL], tdt, kind="ExternalInput")
    zT_d = nc.dram_tensor("zT", [D, BL], f32r, kind="ExternalOutput")
    ldo_d = nc.dram_tensor("ldo", [BL, 1], f32, kind="ExternalOutput")

    with tile.TileContext(nc) as tc:
        with (
            tc.tile_pool(name="const", bufs=1) as cp,
            tc.tile_pool(name="state", bufs=T["sp"]) as sp,
            tc.tile_pool(name="work", bufs=T["wp"]) as wp,
            tc.tile_pool(name="pa", bufs=T["pa"], space="PSUM") as pa,
            tc.tile_pool(name="pt", bufs=T["tp"], space="PSUM") as pt,
            tc.tile_pool(name="psfS", bufs=T["S"], space="PSUM") as psfS,
            tc.tile_pool(name="psfF", bufs=T["F"], space="PSUM") as psfF,
        ):
            # ---- constants ----
            W1e = cp.tile([KU, H], f32r)
            nc.sync.dma_start(W1e[:], W1e_d[:])
            W2s = cp.tile([128, NCH, H], f32r)
            nc.sync.dma_start(W2s[:], W2s_d[:])
            Ms = cp.tile([128, NCH, H], f32r)
            nc.sync.dma_start(Ms[:], Ms_d[:])
            W3p = cp.tile([128, NCH, 4, D], tdt)
            nc.sync.dma_start(W3p[:], W3p_d[:])
            idb = cp.tile([BL, BL], tdt)
            nc.sync.dma_start(idb[:], idb_d[:])
            trows = cp.tile([1, 17 * BL], f32r)
            nc.sync.dma_start(trows[:], trows_d[:])
            if with_b1:
                b1c = cp.tile([128, NCH], f32)
                nc.sync.dma_start(b1c[:], b1c_d[:])
            if with_b2:
                b2r = cp.tile([1, H], f32r)
                nc.sync.dma_start(b2r[:], b2r_d[:])
                onesr = cp.tile([1, BL], f32r)
                nc.sync.dma_start(onesr[:], onesr_d[:])
            if with_b3:
                b3p = cp.tile([1, 4, D], tdt)
                nc.sync.dma_start(b3p[:], b3p_d[:])
                onesb = cp.tile([1, BL], tdt)
                nc.sync.dma_start(onesb[:], onesb_d[:])
            zb = cp.tile([128, 1], f32)
            nc.gpsimd.memset(zb[:], 0.0)
            # per-stage -w*tr accumulator columns
            trbuf = cp.tile([BL, 32], f32)

            # ---- state ----
            ubase = sp.tile([KU, BL], f32r, tag="ubase")
            nc.sync.dma_start(ubase[:], u0_d[:])

            def stage(u_i, t_idx, i, sidx, pF, first_f, last_f):
                """One aug_dyn evaluation. Returns pS (or None)."""
                # t row of u (from the host-side t table)
                nc.vector.tensor_copy(u_i[D:D + 1, :],
                                      trows[0:1, t_idx * BL:(t_idx + 1) * BL])

                # L1 (feature-major): a1T chunks = W1e_chunk^T @ u
                h1T = wp.tile([128, NCH, BL], f32r, tag="h1T")
                for w in range(2):
                    a1T = pt.tile([128, 2, 512], f32, tag="tp")
                    for jj in range(2):
                        j = 2 * w + jj
                        nc.tensor.matmul(a1T[:, jj, 0:BL],
                                         W1e[:, j * 128:(j + 1) * 128],
                                         u_i[:], start=True, stop=True)
                    if with_b1:
                        for jj in range(2):
                            j = 2 * w + jj
                            nc.scalar.activation(
                                h1T[:, j, :], a1T[:, jj, 0:BL], AF.Tanh,
                                bias=b1c[:, j:j + 1])
                    else:
                        nc.scalar.activation(
                            h1T[:, 2 * w:2 * w + 2, :], a1T[:, :, 0:BL],
                            AF.Tanh, bias=zb[:, :])

                # trace g1 path (feature-major): g1wT = w_i * (1 - h1T^2)
                s1T = wp.tile([128, NCH, BL], f32, tag="s1T")
                nc.scalar.activation(s1T[:], h1T[:], AF.Square, bias=zb[:, :])
                g1wT = wp.tile([128, NCH, BL], f32r, tag="g1wT")
                nc.vector.tensor_scalar(g1wT[:], s1T[:], 1.0, -float(LD_W[i]),
                                        OP.subtract, OP.mult)

                # L2: a2 = h1 @ W2 (+ b2)
                a2 = pa.tile([BL, H], f32, tag="a")
                for j in range(NCH):
                    nc.tensor.matmul(a2[:], h1T[:, j, :], W2s[:, j, :],
                                     start=(j == 0),
                                     stop=(j == NCH - 1 and not with_b2))
                if with_b2:
                    nc.tensor.matmul(a2[:], onesr[:], b2r[:],
                                     start=False, stop=True)

                # h2 (tdt), PE-transpose -> h2T
                h2 = wp.tile([BL, H], tdt, tag="h2")
                nc.scalar.activation(h2[:], a2[:], AF.Tanh, bias=zb[0:BL, :])
                h2T = wp.tile([128, NCH, BL], tdt, tag="h2T")
                tpw = 1024 if use_bf16 else 512
                for w in range(2):
                    tp = pt.tile([128, 2, tpw], tdt, tag="tp")
                    for jj in range(2):
                        j = 2 * w + jj
                        nc.tensor.transpose(tp[:, jj, 0:BL],
                                            h2[:, j * 128:(j + 1) * 128],
                                            idb[:])
                    nc.vector.tensor_copy(h2T[:, 2 * w:2 * w + 2, :],
                                          tp[:, :, 0:BL])

                # s2 = h2^2
                s2 = wp.tile([BL, H], tdt, tag="s2")
                nc.vector.tensor_mul(s2[:], h2[:], h2[:])

                # trace matmul: y' = w_i * g1^T M  (batch-major out)
                y = pa.tile([BL, H], f32, tag="a")
                for j in range(NCH):
                    nc.tensor.matmul(y[:], g1wT[:, j, :], Ms[:, j, :],
                                     start=(j == 0), stop=(j == NCH - 1))

                # -w*tr = sum((s2 - 1) * y') into trbuf column
                scr = wp.tile([BL, H], f32, tag="scr")
                nc.vector.scalar_tensor_tensor(
                    scr[:], s2[:], 1.0, y[:], OP.subtract, OP.mult,
                    accum_out=trbuf[:, sidx:sidx + 1])

                # L3 state increment (stages 0-2): pS = c_i * (W3^T h2 + b3)
                pS = None
                if i < 3:
                    sv = S_VAR[i]
                    pS = psfS.tile([D, BL], f32, tag="S")
                    for j in range(NCH):
                        nc.tensor.matmul(pS[:], W3p[:, j, sv, :], h2T[:, j, :],
                                         start=(j == 0),
                                         stop=(j == NCH - 1 and not with_b3))
                    if with_b3:
                        nc.tensor.matmul(pS[:], b3p[0:1, sv, :], onesb[:],
                                         start=False, stop=True)
                # L3 final-sum accumulation: pF += d_i * (W3^T h2 + b3)
                fv = F_VAR[i]
                for j in range(NCH):
                    nc.tensor.matmul(pF[:], W3p[:, j, fv, :], h2T[:, j, :],
                                     start=(first_f and j == 0),
                                     stop=(last_f and j == NCH - 1
                                           and not with_b3))
                if with_b3:
                    nc.tensor.matmul(pF[:], b3p[0:1, fv, :], onesb[:],
                                     start=False, stop=last_f)
                return pS

            for _rep in range(reps):
                for s in range(_NSTEPS):
                    pF = psfF.tile([D, BL], f32, tag="F")
                    u_i = ubase
                    for i in range(4):
                        pS = stage(u_i, 2 * s + (0, 1, 1, 2)[i], i,
                                   4 * s + i, pF,
                                   first_f=(i == 0), last_f=(i == 3))
                        if i < 3:
                            u_n = wp.tile([KU, BL], f32r, tag="ust")
                            nc.vector.tensor_tensor(
                                u_n[0:D, :], ubase[0:D, :], pS[:], OP.add)
                            u_i = u_n
                    ub_new = sp.tile([KU, BL], f32r, tag="ubase")
                    nc.vector.tensor_tensor(
                        ub_new[0:D, :], ubase[0:D, :], pF[:], OP.add)
                    ubase = ub_new

            ld = sp.tile([BL, 1], f32, tag="ld")
            nc.vector.tensor_reduce(ld[:], trbuf[:], mybir.AxisListType.X,
                                    OP.add)
            nc.sync.dma_start(zT_d[:], ubase[0:D, :])
            nc.sync.dma_start(ldo_d[:], ld[:])

    nc.compile()
    return nc


def _get_nc(use_bf16, with_b1, with_b2, with_b3, reps=1, tune=None):
    key = (use_bf16, with_b1, with_b2, with_b3, reps,
           tuple(sorted(tune.items())) if tune else None)
    if key not in _compiled:
        _compiled[key] = _build_nc(use_bf16, with_b1, with_b2, with_b3, reps,
                                   tune)
    return _compiled[key]


def _host_inputs(x, W1, b1, W2, b2, W3, b3, use_bf16, with_b1, with_b2,
                 with_b3):
    import ml_dtypes
    f32 = np.float32
    tnp = ml_dtypes.bfloat16 if use_bf16 else f32
    D, H, BL = _D, _H, _BL
    NCH = H // 128

    C = W3.astype(np.float64) @ W1[:D].astype(np.float64)    # [H, H]
    M = W2.astype(np.float64) * C.T                          # [k1, k2]

    W1e = W1.astype(f32)                                     # [65, H]
    W2s = np.ascontiguousarray(
        W2.reshape(NCH, 128, H).transpose(1, 0, 2)).astype(f32)
    Ms = np.ascontiguousarray(
        M.reshape(NCH, 128, H).transpose(1, 0, 2)).astype(f32)
    scales = [_DT / 2, _DT, _DT / 6, _DT / 3]
    W3v = np.stack([(sc * W3.astype(np.float64)).reshape(NCH, 128, D)
                    for sc in scales], axis=0)               # [4v, NCH, 128, D]
    W3p = np.ascontiguousarray(W3v.transpose(2, 1, 0, 3)).astype(tnp)
    tvals = np.arange(17, dtype=np.float64) * (_DT / 2)
    trows = np.repeat(tvals, BL)[None, :].astype(f32)

    shared = {"W1e": W1e, "W2s": W2s, "Ms": Ms, "W3p": W3p,
              "idb": np.eye(BL, dtype=tnp), "trows": trows}
    if with_b1:
        shared["b1c"] = np.ascontiguousarray(
            b1.reshape(NCH, 128).T).astype(f32)
    if with_b2:
        shared["b2r"] = b2[None, :].astype(f32)
        shared["onesr"] = np.ones((1, BL), f32)
    if with_b3:
        b3v = np.stack([(sc * b3.astype(np.float64)) for sc in scales], 0)
        shared["b3p"] = np.ascontiguousarray(b3v[None, :, :]).astype(tnp)
        shared["onesb"] = np.ones((1, BL), tnp)

    xs = x.reshape(_NCORES, BL, D)
    in_maps = []
    for c in range(_NCORES):
        u0 = np.zeros((D + 1, BL), f32)
        u0[0:D] = xs[c].T
        in_maps.append({"u0": u0, **shared})
    return in_maps


def run(x, W1, b1, W2, b2, W3, b3, use_bf16=True, reps=1, tune=None):
    from concourse.bass_utils import run_bass_kernel_spmd
    x = np.asarray(x, np.float32)
    W1 = np.asarray(W1, np.float32)
    b1 = np.asarray(b1, np.float32)
    W2 = np.asarray(W2, np.float32)
    b2 = np.asarray(b2, np.float32)
    W3 = np.asarray(W3, np.float32)
    b3 = np.asarray(b3, np.float32)
    with_b1 = bool(np.any(b1 != 0.0))
    with_b2 = bool(np.any(b2 != 0.0))
    with_b3 = bool(np.any(b3 != 0.0))
    nc = _get_nc(use_bf16, with_b1, with_b2, with_b3, reps, tune)
    in_maps = _host_inputs(x, W1, b1, W2, b2, W3, b3,
                           use_bf16, with_b1, with_b2, with_b3)
    res = run_bass_kernel_spmd(nc, in_maps, core_ids=list(range(_NCORES)))
    z = np.concatenate([r["zT"].T for r in res.results], axis=0)
    log_det = np.concatenate([r["ldo"][:, 0] for r in res.results], axis=0)
    return z.astype(np.float32), log_det.astype(np.float32)


def kernel(x, W1, b1, W2, b2, W3, b3):
    return run(x, W1, b1, W2, b2, W3, b3)
